# revision 6
# baseline (speedup 1.0000x reference)
"""FFM layer on 8 Trainium2 NeuronCores — conv-hybrid, T-block sharded.

Each core owns a 512-row block of the sequence and produces its block of
the output directly; the only collective is a 1 KB AllGather of scan
carries (fully overlapped with the conv matmuls).

  zm[t,o] = sum_{i,Delta} z[t-Delta, i] * G[(i,Delta), o]
  G[(i,D),o] = rho_i^D * sum_j cos(b_j D) Wre[i,j,o] + sin(b_j D) Wim[i,j,o]

Traces 2..63 (kernel decays within <= 384 steps) go through this causal-
conv-as-matmul with per-trace truncation; traces 0..1 (slow decay) use the
rotated real-scan pair (C,S) over the local block plus a carry correction
C' = C + rho^{tau+1} * I_c, where I_c is a weighted sum of the other
cores' block-end columns E (AllGather of [128,2] fp32). E is computed
directly as a weighted reduction of the scan INPUT (accum_out of a fused
multiply), so the collective launches before the scans even finish.

Schedule: sync queue carries the latency-critical small DMAs; the DVE
queue streams im2col (diagonal DRAM->SBUF views of z) interleaved with
the G table; PE runs A -> per-psum conv sweeps (stagger) with B-prep
interleaved, so each psum's LayerNorm overlaps the next psum's matmuls.
"""

import numpy as np
from contextlib import ExitStack

import concourse.bacc as bacc
import concourse.bass as bass
import concourse.tile as tile
from concourse import mybir
from concourse.bass_utils import run_bass_kernel_spmd

T, IN, TR, CTX, OUT = 4096, 512, 64, 64, 512
NCORES = 8
BLK = T // NCORES       # 512 rows per core
NSCAN = 2               # traces handled by scan
LN_EPS = 1e-6
FP32 = mybir.dt.float32
BF16 = mybir.dt.bfloat16
AOT = mybir.AluOpType
AFT = mybir.ActivationFunctionType

# conv plan: per-trace entries (trace, L) with L multiple of 128, then
# packed classes (first_trace, n_traces, L) with 128//L traces per chunk.
PER_TRACE = [(2, 384), (3, 256), (4, 256), (5, 256),
             (6, 128), (7, 128), (8, 128), (9, 128)]
PACKED = [(10, 12, 64), (22, 24, 32), (46, 18, 16)]

_CACHE: dict = {}


def _conv_plan():
    """entries for DMA generation + flat row map [(trace, delta)], -1=pad."""
    entries = []
    rowmap = []
    c0 = 0
    for i, L in PER_TRACE:
        k = L // 128
        entries.append(("per_trace", i, L, c0, k))
        block = np.full((k * 128, 2), (-1, 0), np.int64)
        for cc in range(k):
            for p in range(128):
                dp = p * k + cc
                block[cc * 128 + p] = (i, L - 1 - dp)
        rowmap.append(block)
        c0 += k
    for i0, nt, L in PACKED:
        tpc = 128 // L
        nch = (nt + tpc - 1) // tpc
        entries.append(("packed", i0, nt, L, c0, nch))
        block = np.full((nch * 128, 2), (-1, 0), np.int64)
        for cc in range(nch):
            for h in range(tpc):
                tr = i0 + tpc * cc + h
                if tr >= i0 + nt:
                    continue
                for dpr in range(L):
                    block[cc * 128 + h * L + dpr] = (tr, L - 1 - dpr)
        rowmap.append(block)
        c0 += nch
    return entries, np.concatenate(rowmap), c0


CONV_ENTRIES, ROWMAP, NCH = _conv_plan()
KCONV = NCH * 128


def _ap(t: bass.AP, col_off: int, dims) -> bass.AP:
    """AP over an SBUF tile slice: keep its partition dim, custom free dims."""
    return bass.AP(tensor=t.tensor, offset=t.offset + col_off,
                   ap=[t.ap[0]] + list(dims))


def _free_bcast(col: bass.AP, n: int) -> bass.AP:
    return bass.AP(tensor=col.tensor, offset=col.offset,
                   ap=[col.ap[0], [0, n]])


def _build_module(with_state0: bool = False):
    nc = bacc.Bacc("TRN2", target_bir_lowering=False, debug=False,
                   num_devices=NCORES)

    def inp(name, shape, dt):
        return nc.dram_tensor(name, list(shape), dt, kind="ExternalInput").ap()

    xT_in = inp("xT_in", (IN, 2 * BLK), BF16)        # [prev block | own block]^T
    wpg = inp("wpg", (IN, 128), BF16)                # [W_pre | W_gin] columns
    trig = inp("trig", (128, 4 * BLK), BF16)         # cos|sin|rhopow|rhopow_rev
    mcol = inp("mcol", (128, 24), FP32)              # rho01,wgt_rep,s0term,mask,bias
    wmix_sc = inp("wmix_sc", (2 * 128, OUT), BF16)   # scan-trace mix rows (re|im)
    wgs = inp("wgs", (8 * 128, OUT), BF16)           # gout 4 chunks | skip 4 chunks
    gtab = inp("gtab", (KCONV, OUT), BF16)           # conv kernel table
    ones_row = inp("ones_row", (1, 128), BF16)
    brow = inp("brow", (1, 3 * OUT), BF16)           # bgout | bskip | bmix

    outc = nc.dram_tensor("outc", [BLK, OUT], FP32, kind="ExternalOutput").ap()
    groups = [list(range(NCORES))]

    with tile.TileContext(nc) as tc, ExitStack() as ctx:
        const = ctx.enter_context(tc.tile_pool(name="const", bufs=1))
        dram = ctx.enter_context(tc.tile_pool(name="dram", bufs=1, space="DRAM"))

        # ---- latency-critical loads (sync queue, in need-order) ----------
        xt = const.tile([128, 4 * 2 * BLK], BF16)    # (IN-chunk, [prev|own] t)
        for h in (1, 0):                             # own half first
            nc.sync.dma_start(
                _ap(xt, h * BLK, [[2 * BLK, 4], [1, BLK]]),
                bass.AP(tensor=xT_in.tensor, offset=h * BLK,
                        ap=[[2 * BLK, 128], [128 * 2 * BLK, 4], [1, BLK]]))
        wpg_sb = const.tile([128, 4 * 128], BF16)
        nc.sync.dma_start(
            wpg_sb, bass.AP(tensor=wpg.tensor, offset=0,
                            ap=[[128, 128], [128 * 128, 4], [1, 128]]))
        trig_sb = const.tile([128, 4 * BLK], BF16)
        nc.sync.dma_start(trig_sb, trig)
        cosb = trig_sb[:, 0:BLK]
        sinb = trig_sb[:, BLK:2 * BLK]
        rhopow = trig_sb[:, 2 * BLK:3 * BLK]
        rhoprev = trig_sb[:, 3 * BLK:4 * BLK]
        mcol_sb = const.tile([128, 24], FP32)
        nc.sync.dma_start(mcol_sb, mcol)
        eps_sb = const.tile([128, 1], FP32)
        nc.vector.memset(eps_sb, LN_EPS)

        zD = dram.tile([TR, 2 * BLK], BF16, name="zD")
        E_my = dram.tile([128, 2], FP32, name="E_my")
        E_all = dram.tile([128 * NCORES, 2], FP32, name="E_all")

        # ---- A: gated z, own block first (feeds the scan/E chain) --------
        zb = const.tile([128, BLK], BF16)
        with tc.tile_pool(name="psa", bufs=2, space="PSUM") as psa:
            for h in (1, 0):
                ps = psa.tile([128, BLK], FP32, tag="za", bufs=2)
                for ki in range(4):
                    nc.tensor.matmul(
                        ps, wpg_sb[:, ki * 128:(ki + 1) * 128],
                        xt[:, ki * 2 * BLK + h * BLK: ki * 2 * BLK + (h + 1) * BLK],
                        start=(ki == 0), stop=(ki == 3))
                pre_sb = const.tile([64, BLK], FP32, tag=f"pre{h}")
                nc.scalar.activation(pre_sb, ps[0:64, :], AFT.Identity,
                                     bias=mcol_sb[0:64, 23:24])
                sig_sb = const.tile([64, BLK], FP32, tag=f"sig{h}")
                nc.scalar.activation(sig_sb, ps[64:128, :], AFT.Sigmoid,
                                     bias=mcol_sb[64:128, 23:24])
                zt = const.tile([64, BLK], BF16, tag=f"z{h}")
                if h == 0:   # prev block: masked to 0 on core 0
                    nc.vector.scalar_tensor_tensor(
                        zt, pre_sb, mcol_sb[0:64, 20:21], sig_sb,
                        op0=AOT.mult, op1=AOT.mult)
                else:
                    nc.vector.tensor_mul(zt, pre_sb, sig_sb)
                nc.sync.dma_start(
                    bass.AP(tensor=zD.tensor, offset=zD.offset + h * BLK,
                            ap=[[2 * BLK, TR], [1, BLK]]), zt)
                if h == 1:   # broadcast own-block z for the scan traces now
                    for il in range(NSCAN):
                        nc.sync.dma_start(
                            zb[il * CTX:(il + 1) * CTX, :],
                            bass.AP(tensor=zD.tensor,
                                    offset=zD.offset + il * 2 * BLK + BLK,
                                    ap=[[0, CTX], [1, BLK]]))

        # remaining const loads (sync queue, after the critical z path)
        wmix_sb = const.tile([128, 2 * OUT], BF16)
        nc.sync.dma_start(
            wmix_sb, bass.AP(tensor=wmix_sc.tensor, offset=0,
                             ap=[[OUT, 128], [128 * OUT, 2], [1, OUT]]))
        wgs_sb = const.tile([128, 8 * OUT], BF16)
        nc.sync.dma_start(
            wgs_sb, bass.AP(tensor=wgs.tensor, offset=0,
                            ap=[[OUT, 128], [128 * OUT, 8], [1, OUT]]))
        ones_sb = const.tile([1, 128], BF16)
        nc.sync.dma_start(ones_sb, ones_row)
        brow_sb = const.tile([1, 3 * OUT], BF16)
        nc.sync.dma_start(brow_sb, brow)

        # ---- scan traces: inputs, E columns (no scan needed!), scans -----
        cc_t = const.tile([128, BLK], BF16)
        nc.vector.tensor_mul(cc_t, zb, cosb)
        ss_t = const.tile([128, BLK], BF16)
        nc.vector.tensor_mul(ss_t, zb, sinb)
        E_sb = const.tile([128, 2], FP32)
        scr = const.tile([128, BLK], BF16)           # discard target
        nc.vector.scalar_tensor_tensor(
            scr, rhoprev, 1.0, cc_t, op0=AOT.mult, op1=AOT.mult,
            accum_out=E_sb[:, 0:1])
        scr2 = const.tile([128, BLK], BF16)
        nc.vector.scalar_tensor_tensor(
            scr2, rhoprev, 1.0, ss_t, op0=AOT.mult, op1=AOT.mult,
            accum_out=E_sb[:, 1:2])
        C_t = const.tile([128, BLK], BF16)
        nc.vector.tensor_tensor_scan(
            C_t, _free_bcast(mcol_sb[:, 0:1], BLK), cc_t, initial=0.0,
            op0=AOT.mult, op1=AOT.add)
        S_t = const.tile([128, BLK], BF16)
        nc.vector.tensor_tensor_scan(
            S_t, _free_bcast(mcol_sb[:, 0:1], BLK), ss_t, initial=0.0,
            op0=AOT.mult, op1=AOT.add)

        # E exchange entirely on the gpsimd queue (no head-of-line blocking)
        nc.gpsimd.dma_start(E_my, E_sb)
        nc.gpsimd.collective_compute(
            "AllGather", AOT.bypass, replica_groups=groups,
            ins=[E_my.opt()], outs=[E_all.opt()])
        E_all_sb = const.tile([128, 16], FP32)
        nc.gpsimd.dma_start(
            E_all_sb,
            bass.AP(tensor=E_all.tensor, offset=E_all.offset,
                    ap=[[2, 128], [256, NCORES], [1, 2]]))

        # ---- im2col + G table, interleaved on the ACT HWDGE queue --------------
        imcol = const.tile([128, NCH * BLK], BF16)
        g_sb = const.tile([128, NCH * OUT], BF16)
        gq = [0, 7, 14, 21, NCH]     # G quarter boundaries (chunk index)

        def load_g(q):
            h0, nh = gq[q], gq[q + 1] - gq[q]
            nc.scalar.dma_start(
                _ap(g_sb, h0 * OUT, [[OUT, nh], [1, OUT]]),
                bass.AP(tensor=gtab.tensor, offset=h0 * 128 * OUT,
                        ap=[[OUT, 128], [128 * OUT, nh], [1, OUT]]))

        gq_next = 0

        def maybe_g(c_done):
            nonlocal gq_next
            while gq_next < 4 and gq[gq_next] <= c_done:
                load_g(gq_next)
                gq_next += 1

        maybe_g(0)
        for e in CONV_ENTRIES:
            if e[0] == "per_trace":
                _, i, L, c0, k = e
                nc.scalar.dma_start(
                    _ap(imcol, c0 * BLK, [[BLK, k], [1, BLK]]),
                    bass.AP(tensor=zD.tensor,
                            offset=zD.offset + i * 2 * BLK + BLK + 1 - L,
                            ap=[[k, 128], [1, k], [1, BLK]]))
                maybe_g(c0 + k)
            else:
                _, i0, nt, L, c0, nch = e
                tpc = 128 // L
                for h in range(tpc):
                    nch_h = (nt - h + tpc - 1) // tpc
                    base = imcol[h * L:(h + 1) * L, :]
                    nc.scalar.dma_start(
                        bass.AP(tensor=base.tensor,
                                offset=base.offset + c0 * BLK,
                                ap=[base.ap[0], [BLK, nch_h], [1, BLK]]),
                        bass.AP(tensor=zD.tensor,
                                offset=(zD.offset + (i0 + h) * 2 * BLK
                                        + BLK + 1 - L),
                                ap=[[1, L], [tpc * 2 * BLK, nch_h], [1, BLK]]))
                npad = nch * 128 - ((nt - 1) // tpc) * 128 - \
                    ((nt - 1) % tpc + 1) * L
                if npad > 0:   # ragged tail: fill with dup rows (G=0)
                    base = imcol[128 - npad:128, :]
                    nc.scalar.dma_start(
                        bass.AP(tensor=base.tensor,
                                offset=base.offset + (c0 + nch - 1) * BLK,
                                ap=[base.ap[0], [1, BLK]]),
                        bass.AP(tensor=zD.tensor,
                                offset=zD.offset + (TR - 1) * 2 * BLK + BLK,
                                ap=[[0, npad], [1, BLK]]))
                maybe_g(c0 + nch)

        # ---- carry correction + rotate-back (DVE, after AllGather) -------
        prod = const.tile([128, 16], FP32)
        nc.vector.tensor_mul(prod, E_all_sb, mcol_sb[:, 2:18])
        f1 = const.tile([128, 8], FP32)
        nc.vector.tensor_add(f1, prod[:, 0:8], prod[:, 8:16])
        f2 = const.tile([128, 4], FP32)
        nc.vector.tensor_add(f2, f1[:, 0:4], f1[:, 4:8])
        icis = const.tile([128, 2], FP32)
        if with_state0:
            f3 = const.tile([128, 2], FP32)
            nc.vector.tensor_add(f3, f2[:, 0:2], f2[:, 2:4])
            nc.vector.tensor_add(icis, f3, mcol_sb[:, 18:20])
        else:
            nc.vector.tensor_add(icis, f2[:, 0:2], f2[:, 2:4])
        Cc = const.tile([128, BLK], BF16)
        nc.vector.scalar_tensor_tensor(
            Cc, rhopow, icis[:, 0:1], C_t, op0=AOT.mult, op1=AOT.add)
        Sc = const.tile([128, BLK], BF16)
        nc.vector.scalar_tensor_tensor(
            Sc, rhopow, icis[:, 1:2], S_t, op0=AOT.mult, op1=AOT.add)
        m1 = const.tile([128, BLK], BF16)
        nc.vector.tensor_mul(m1, Cc, cosb)
        m2 = const.tile([128, BLK], BF16)
        nc.vector.tensor_mul(m2, Sc, sinb)
        s_r = const.tile([128, BLK], BF16)
        nc.vector.tensor_add(s_r, m1, m2)
        m3 = const.tile([128, BLK], BF16)
        nc.vector.tensor_mul(m3, Cc, sinb)
        m4 = const.tile([128, BLK], BF16)
        nc.vector.tensor_mul(m4, Sc, cosb)
        s_i = const.tile([128, BLK], BF16)
        nc.vector.tensor_sub(s_i, m3, m4)

        # ---- PE: staggered per-psum conv sweeps + B-prep interleave ------
        gout_st = const.tile([128, 4 * OUT], BF16)
        skip_st = const.tile([128, 4 * OUT], BF16)
        t2_st = const.tile([128, 4 * OUT], BF16)

        with tc.tile_pool(name="psz", bufs=1, space="PSUM") as psz, \
                tc.tile_pool(name="psb", bufs=2, space="PSUM") as psb, \
                tc.tile_pool(name="pb", bufs=2) as pb:
            zmps = [psz.tile([128, OUT], FP32, tag=f"zm{i}", name=f"zm{i}")
                    for i in range(4)]

            def bprep(tc4):
                toff = 512 + tc4 * 128
                osl = slice(tc4 * OUT, (tc4 + 1) * OUT)
                ps_go = psb.tile([128, OUT], FP32, tag="go", bufs=2)
                for ki in range(4):
                    nc.tensor.matmul(
                        ps_go,
                        xt[:, ki * 2 * BLK + toff: ki * 2 * BLK + toff + 128],
                        wgs_sb[:, ki * OUT:(ki + 1) * OUT],
                        start=(ki == 0), stop=False)
                nc.tensor.matmul(ps_go, ones_sb, brow_sb[:, 0:OUT],
                                 start=False, stop=True)
                nc.scalar.activation(gout_st[:, osl], ps_go, AFT.Sigmoid)
                ps_sk = psb.tile([128, OUT], FP32, tag="sk", bufs=2)
                for ki in range(4):
                    nc.tensor.matmul(
                        ps_sk,
                        xt[:, ki * 2 * BLK + toff: ki * 2 * BLK + toff + 128],
                        wgs_sb[:, (4 + ki) * OUT:(5 + ki) * OUT],
                        start=(ki == 0), stop=False)
                nc.tensor.matmul(ps_sk, ones_sb, brow_sb[:, OUT:2 * OUT],
                                 start=False, stop=True)
                nc.scalar.copy(skip_st[:, osl], ps_sk)
                # t2 = (gout-1)*skip precomputed off the critical B path
                nc.gpsimd.scalar_tensor_tensor(
                    t2_st[:, osl], gout_st[:, osl], 1.0, skip_st[:, osl],
                    op0=AOT.subtract, op1=AOT.mult)

            def bphase(tc4):
                osl = slice(tc4 * OUT, (tc4 + 1) * OUT)
                zm_sb = pb.tile([128, OUT], BF16, tag="zm_sb")
                nc.scalar.copy(zm_sb, zmps[tc4])
                v = pb.tile([128, OUT], BF16, tag="v")
                nc.vector.tensor_mul(v, zm_sb, gout_st[:, osl])
                stats = pb.tile([128, 6], FP32, tag="stats")
                nc.vector.bn_stats(stats, v)
                mv = pb.tile([128, 2], FP32, tag="mv")
                nc.vector.bn_aggr(mv, stats)
                sd = pb.tile([128, 1], FP32, tag="sd")
                nc.scalar.activation(sd, mv[:, 1:2], AFT.Sqrt, bias=eps_sb)
                rstd = pb.tile([128, 1], FP32, tag="rstd")
                nc.vector.reciprocal(rstd, sd)
                ln = pb.tile([128, OUT], BF16, tag="ln")
                nc.vector.tensor_scalar(
                    ln, v, mv[:, 0:1], rstd, op0=AOT.subtract, op1=AOT.mult)
                res = pb.tile([128, OUT], FP32, tag="res")
                nc.vector.tensor_sub(res, ln, t2_st[:, osl])
                nc.gpsimd.dma_start(outc[tc4 * 128:(tc4 + 1) * 128, :], res)

            for tc4 in range(4):
                for c in range(NCH):
                    nc.tensor.matmul(
                        zmps[tc4],
                        imcol[:, c * BLK + tc4 * 128: c * BLK + tc4 * 128 + 128],
                        g_sb[:, c * OUT:(c + 1) * OUT],
                        start=(c == 0), stop=False)
                    # fill the DMA-paced first sweep with B-prep matmuls
                    if tc4 == 0 and c in (2, 7, 12, 17):
                        bprep((2, 7, 12, 17).index(c))
                nc.tensor.matmul(zmps[tc4], ones_sb, brow_sb[:, 2 * OUT:3 * OUT],
                                 start=False, stop=False)
                nc.tensor.matmul(
                    zmps[tc4], s_r[:, tc4 * 128:(tc4 + 1) * 128],
                    wmix_sb[:, 0:OUT], start=False, stop=False)
                nc.tensor.matmul(
                    zmps[tc4], s_i[:, tc4 * 128:(tc4 + 1) * 128],
                    wmix_sb[:, OUT:2 * OUT], start=False, stop=True)
                bphase(tc4)

    nc.compile()
    return nc


def _prep_inputs(inputs):
    x = np.asarray(inputs["x"], np.float32)
    state0 = np.asarray(inputs["state0"], np.float64)
    a = np.abs(np.asarray(inputs["ffa_a"], np.float64))
    b = np.asarray(inputs["ffa_b"], np.float64)
    rho = np.exp(-a)
    W_pre = np.asarray(inputs["W_pre"], np.float32)
    b_pre = np.asarray(inputs["b_pre"], np.float32)
    W_gin = np.asarray(inputs["W_gin"], np.float32)
    b_gin = np.asarray(inputs["b_gin"], np.float32)
    W_gout = np.asarray(inputs["W_gout"], np.float32)
    b_gout = np.asarray(inputs["b_gout"], np.float32)
    W_skip = np.asarray(inputs["W_skip"], np.float32)
    b_skip = np.asarray(inputs["b_skip"], np.float32)
    W_mix = np.asarray(inputs["W_mix"], np.float64)
    b_mix = np.asarray(inputs["b_mix"], np.float32)
    Wm = W_mix.reshape(TR, 2, CTX, OUT)

    bf16 = mybir.dt.np(BF16)

    # G table (same for all cores)
    G = np.zeros((KCONV, OUT), np.float32)
    for i in range(NSCAN, TR):
        rows = np.nonzero(ROWMAP[:, 0] == i)[0]
        if len(rows) == 0:
            continue
        ds = ROWMAP[rows, 1].astype(np.float64)
        ang = np.outer(ds, b)
        G[rows] = ((np.cos(ang) @ Wm[i, 0] + np.sin(ang) @ Wm[i, 1])
                   * (rho[i] ** ds)[:, None]).astype(np.float32)
    G = G.astype(bf16)

    wpg_h = np.concatenate([W_pre, W_gin], axis=1).astype(bf16)   # (512,128)
    wgs_h = np.concatenate([W_gout.reshape(4, 128, OUT),
                            W_skip.reshape(4, 128, OUT)], axis=0) \
        .reshape(8 * 128, OUT).astype(bf16)
    wmix_h = np.concatenate(
        [Wm[0:NSCAN, 0].reshape(128, OUT),
         Wm[0:NSCAN, 1].reshape(128, OUT)], axis=0).astype(bf16)
    ones_h = np.ones((1, 128), bf16)
    brow_h = np.concatenate([b_gout, b_skip, b_mix])[None, :].astype(bf16)

    jj = np.tile(np.arange(CTX), 2)                 # j per partition
    ii = np.repeat(np.arange(NSCAN), CTX)           # trace per partition
    tau = np.arange(BLK, dtype=np.float64)
    rhopow_h = rho[ii][:, None] ** (tau[None, :] + 1.0)
    rhoprev_h = rho[ii][:, None] ** (BLK - 1.0 - tau[None, :])

    s0c = state0[0, :, :, 0] + 1j * state0[0, :, :, 1]   # (TR, CTX)
    r_init = np.exp(1j * b)[None, :] * s0c[0:NSCAN]      # R_{-1} per (il,j)
    initC = r_init.real.reshape(-1)
    initS = (-r_init.imag).reshape(-1)

    xb = x.astype(bf16)
    in_maps = []
    for c in range(NCORES):
        t0 = c * BLK
        xT_h = np.zeros((IN, 2 * BLK), bf16)
        if c > 0:
            xT_h[:, 0:BLK] = xb[t0 - BLK:t0].T
        xT_h[:, BLK:] = xb[t0:t0 + BLK].T

        tg = (t0 + np.arange(BLK, dtype=np.float64))[None, :]
        ang = b[jj][:, None] * tg                    # (128, BLK)
        trig_h = np.concatenate(
            [np.cos(ang), np.sin(ang), rhopow_h, rhoprev_h],
            axis=1).astype(bf16)

        mcol_h = np.zeros((128, 24), np.float32)
        mcol_h[:, 0] = rho[ii]
        for bb in range(c):
            w = rho[ii] ** (512.0 * (c - 1 - bb))
            mcol_h[:, 2 + 2 * bb] = w
            mcol_h[:, 2 + 2 * bb + 1] = w
        mcol_h[:, 18] = (rho[ii] ** (512.0 * c)) * initC
        mcol_h[:, 19] = (rho[ii] ** (512.0 * c)) * initS
        mcol_h[0:64, 20] = 0.0 if c == 0 else 1.0
        mcol_h[0:64, 23] = b_pre
        mcol_h[64:128, 23] = b_gin

        in_maps.append({
            "xT_in": xT_h,
            "wpg": wpg_h,
            "trig": trig_h,
            "mcol": mcol_h,
            "wmix_sc": wmix_h,
            "wgs": wgs_h,
            "gtab": G,
            "ones_row": ones_h,
            "brow": brow_h,
        })
    return in_maps


def _assemble(results) -> np.ndarray:
    return np.concatenate(
        [np.asarray(results[c]["outc"]) for c in range(NCORES)], axis=0)


def _get_module(with_state0: bool = False):
    key = f"m{int(with_state0)}"
    if key not in _CACHE:
        _CACHE[key] = _build_module(with_state0)
    return _CACHE[key]


def kernel(**inputs) -> np.ndarray:
    with_s0 = bool(np.any(np.asarray(inputs["state0"])))
    nc = _get_module(with_s0)
    in_maps = _prep_inputs(inputs)
    res = run_bass_kernel_spmd(nc, in_maps, list(range(NCORES)))
    return _assemble(res.results)


if __name__ == "__main__":
    import reference
    inputs = reference.setup_inputs()
    out = kernel(**{k: np.asarray(v) for k, v in inputs.items()})
    print("kernel output", out.shape, out.dtype)
